# revision 26
# baseline (speedup 1.0000x reference)
"""Trainium2 Bass kernel for nn_DQSN (dense_mlp spiking network).

Math: the reference runs T=16 steps of an IF neuron driven by a constant
input h = x@w1.T + b1, hard-reset to exactly 0 on fire, then a linear
readout into a leaky (NonSpikingLIF) accumulator.  Because the drive is
constant and the reset is exact, the spike train is periodic with period
n = ceil(1/h) and the LIF state telescopes to

    v_lif_T = S @ w2.T + (1 - 2^-16) * b2,
    S(h)    = (2^(n*F) - 1) * 2^-17 / (1 - 2^-n),   F = floor(16/n)
            = 0 for h < t_16 (= 0.0625) or h <= 0,
    n       = ceil(1/h) in {1..16}.

The closed form is evaluated per element in 4 DVE ops + 3 ScalarE ops
(instead of 16 threshold compares + add tree, which saturated DVE and
GpSimd through their shared SBUF ports):

    y  = Prelu(ps + b1, alpha=-1e-6)        ScalarE  (negatives -> tiny+)
    r  = recip_approx_fast(y)               DVE      (~51 ULP)
    n  = RN(select(r>=16.0001, 1000, r)     DVE      (+0.5-eps + 2^23
             + 0.49993896) via 2^23 magic             round-to-int trick)
    x  = Exp(-ln2 * n) = 2^-n               ScalarE
    nf = 16 - mod(16, n) = n*floor(16/n)    DVE
    B  = Exp(ln2*nf - 17ln2) = 2^(nf-17)    ScalarE
    S  = B*(1+x)(1+x^2)(1+x^4) -> fp16      DVE      (= B/(1-x) + O(x^8))

The kill value n=1000 makes x underflow to 0 and nf = 16-16 = 0, so
S = 2^-17 ~ 0 exactly with no extra gating.  h <= 0 maps through the
Prelu to a tiny positive whose reciprocal is huge, taking the same kill
branch.  Total error vs the bit-exact staircase is ~0.4% (dominated by
the n=1 geometric-series truncation), well inside the 2e-2 gate.

Phase A (h = w1 @ x.T + b1) keeps the fp16 split 3-product matmul
(wh.xh + wh.xl + wl.xh, ~2^-22 residual); phase C is a plain fp16
matmul of w2 @ S.T with the scaled bias fused into the PSUM eviction.
Data-parallel over 8 cores, 1024 batch rows per core, feature-major.
"""

import numpy as np

import concourse.bass as bass
import concourse.mybir as mybir
from concourse import bacc
from concourse import dve_ops as _dvo
from concourse.bass_utils import run_bass_kernel_spmd
from concourse.dve_spec import (
    C0, C1, C2, C3, One, Spec, Src0, Src1, select, sq,
    _has_src1, _spill_c3_to_src1, lower as _dve_lower,
)
from concourse.dve_uop import DveOpSpec
from concourse.tile import TileContext

P = 128
B = 8192
I_DIM = 256
H_DIM = 1024
O_DIM = 256
T_STEPS = 16
N_CORES = 8
B_LOC = B // N_CORES        # 1024 batch rows per core
KT = I_DIM // P             # 2 k-tiles for phase A
HT = H_DIM // P             # 8 h-tiles
OT = O_DIM // P             # 2 o-tiles
NH = 512                    # matmul free-dim half (one PSUM bank of fp32)

F32 = mybir.dt.float32
F16 = mybir.dt.float16

LN2 = float(np.log(2.0))
MAGIC = float(np.float32(2.0 ** 23))
MAGIC27 = float(np.float32(2.0 ** 27))
RND_OFF = 0.49993896484375      # 0.5 - 2^-14: exact-integer r rounds down
CLAMP = 18.2                    # min-clamp: dead r (>16.0001) -> n in 17..19,
                                # where floor(16/n) = 0 kills via B = 2^-17
SEED16 = -0.23549792 * 16.0     # recip bitnot-seed const, x16 for 16/n
FLOOR_SPILL = -7.09375          # -16 * 0.443359375 floor offset (verified
                                # exact for all n in 1..20 incl. kill range)
PRELU_ALPHA = -1e-6


# ----------------------- custom DVE ops (import-time) ------------------- #

def _register(name, body, ref):
    for op in _dvo.OPS:
        if op.name == name:
            return op
    body = _spill_c3_to_src1(body)
    spec = Spec(body=body, reference=ref)
    row = _dvo._CUSTOM_DVE_ROW_BASE + len(_dvo.OPS)
    shas = {}
    for ver in ("v3", "v4"):
        s = DveOpSpec(name=name, opcode=row, uops=_dve_lower(spec, ver=ver),
                      rd1_en=_has_src1(spec))
        shas[ver] = s.sha(ver)
    op = _dvo.DveOp(name, spec, subdim=False, uops_sha=shas)
    _dvo.OPS.append(op)
    _dvo._SUB_OPCODE_FOR_NAME[name] = row
    _dvo.CUSTOM_DVE_SPECS[name] = spec
    return op


from concourse.dve_spec import AluOp as _AluOp, Bin as _Bin, minn as _minn

# n = ((min(r, 18.2) + 0.49994) + 2^23) - 2^23: round-to-int with clamp.
# Dead r (> 16.00006: h < t_16 or h <= 0 via Prelu) lands in n in {17..19}
# where the floor op below yields F = 0 and B underflows the output to ~0.
MAGIC_N = _register(
    "ANT_MAGIC_MIN",
    (((_minn(Src0, C0) + C2) + C1) - C1),
    lambda in0, in1, s0, s1, imm2: (
        (np.minimum(in0, np.float32(s0)).astype(np.float32)
         + np.float32(imm2) + np.float32(s1)) - np.float32(s1)),
)

# F16 = 16*floor(16/n): bitnot reciprocal seed scaled x16, one Newton step
# computed as m = z0*(32 - n*z0) ~ 256/n (error one-sided, in [-0.35%, 0]),
# then floor via RN((m - 7.09375) + 2^27) - 2^27 (2^27 spacing = 16).
# Exact for every reachable n (host-verified 1..20).  Src1 spills -7.09375.
_z0 = _Bin(_AluOp.BITWISE_NOT, Src0, Src0) * C0
_mm = _z0 * (C1 - Src0 * _z0)


def _ref_floor16f(in0, in1, s0, s1, imm2):
    nx = (~in0.view(np.int32)).view(np.float32)
    z0 = nx * np.float32(s0)
    m = z0 * (np.float32(s1) - in0 * z0)
    m2 = (m + in1.reshape(-1, 1)).astype(np.float32)
    return (m2 + np.float32(imm2)).astype(np.float32) - np.float32(imm2)


FLOOR16F = _register(
    "ANT_FLOOR16F",
    ((((_mm + C3) + C2) - C2)),
    _ref_floor16f,
)

# S = ((B + B*x) * (1+x^2)) * (1+x^4); Src0 = x, Src1 = B; 8 ALU ops
_x2 = sq(Src0)
_x4 = sq(_x2)
POLY_MUL = _register(
    "ANT_POLY_MUL",
    (((Src1 + Src1 * Src0) * (One + _x2)) * (One + _x4)),
    lambda in0, in1, s0, s1, imm2: (
        (in1 + in1 * in0) * (1 + in0 * in0) * (1 + in0 ** 4)),
)


# ----------------------------- bass program ----------------------------- #

def _build_nc() -> bacc.Bacc:
    nc = bacc.Bacc(trn_type="TRN2")

    # x split: [P, bh, kt, NH] host-packed so each bh half is one
    # contiguous-per-partition DMA; w1 split: [P, ht, kt, 128] host-packed
    # so each ht chunk is a small early DMA and A(0) starts ~7us sooner.
    xth_d = nc.dram_tensor("xth", [P, 2 * KT * NH], F16, kind="ExternalInput")
    xtl_d = nc.dram_tensor("xtl", [P, 2 * KT * NH], F16, kind="ExternalInput")
    w1th_d = nc.dram_tensor("w1th", [P, HT * KT * P], F16, kind="ExternalInput")
    w1tl_d = nc.dram_tensor("w1tl", [P, HT * KT * P], F16, kind="ExternalInput")
    b1_d = nc.dram_tensor("b1c", [P, HT], F32, kind="ExternalInput")
    w2t_d = nc.dram_tensor("w2t", [H_DIM, O_DIM], F16, kind="ExternalInput")
    b2_d = nc.dram_tensor("b2c", [P, OT], F32, kind="ExternalInput")
    cc_d = nc.dram_tensor("cc", [P, 3], F32, kind="ExternalInput")
    out_d = nc.dram_tensor("outT", [O_DIM, B_LOC], F16, kind="ExternalOutput")

    ident = mybir.ActivationFunctionType.Identity
    Exp = mybir.ActivationFunctionType.Exp
    Prelu = mybir.ActivationFunctionType.Prelu

    with TileContext(nc) as tc:
        with (
            tc.tile_pool(name="const", bufs=1) as cpool,
            tc.tile_pool(name="state", bufs=1) as spool,
            tc.tile_pool(name="chain", bufs=3) as hpool,
            tc.tile_pool(name="psA", bufs=2, space="PSUM") as ppoolA,
            tc.tile_pool(name="psC", bufs=1, space="PSUM") as ppoolC,
        ):
            # x tiles [P, bh, kt, NH]; w1 tiles [P, ht, kt, 128].  Chunked
            # DMAs ordered so tile-0 inputs land first: A(0) needs only
            # w1*[ht=0] + x*[bh=0]; the hi product runs first so the lo
            # chunks may trail by one DMA.
            xth = cpool.tile([P, 2, KT, NH], F16)
            xtl = cpool.tile([P, 2, KT, NH], F16)
            w1th = cpool.tile([P, HT, KT, P], F16)
            w1tl = cpool.tile([P, HT, KT, P], F16)
            xr = xth_d.ap().rearrange("p (bh k) -> p bh k", bh=2)
            xlr = xtl_d.ap().rearrange("p (bh k) -> p bh k", bh=2)
            w1r = w1th_d.ap().rearrange("p (ht k) -> p ht k", ht=HT)
            w1lr = w1tl_d.ap().rearrange("p (ht k) -> p ht k", ht=HT)
            b1 = cpool.tile([P, HT], F32)
            nc.sync.dma_start(b1[:], b1_d.ap())
            cc = cpool.tile([P, 3], F32)
            nc.sync.dma_start(cc[:], cc_d.ap())
            nc.scalar.dma_start(w1th[:, 0], w1r[:, 0].rearrange("p (kt q) -> p kt q", kt=KT))
            nc.sync.dma_start(xth[:, 0], xr[:, 0].rearrange("p (kt q) -> p kt q", kt=KT))
            nc.scalar.dma_start(w1tl[:, 0], w1lr[:, 0].rearrange("p (kt q) -> p kt q", kt=KT))
            nc.sync.dma_start(xtl[:, 0], xlr[:, 0].rearrange("p (kt q) -> p kt q", kt=KT))
            nc.scalar.dma_start(
                w1th[:, 1:], w1r[:, 1:].rearrange("p ht (kt q) -> p ht kt q", kt=KT))
            nc.sync.dma_start(xth[:, 1], xr[:, 1].rearrange("p (kt q) -> p kt q", kt=KT))
            nc.scalar.dma_start(
                w1tl[:, 1:], w1lr[:, 1:].rearrange("p ht (kt q) -> p ht kt q", kt=KT))
            nc.sync.dma_start(xtl[:, 1], xlr[:, 1].rearrange("p (kt q) -> p kt q", kt=KT))
            w2t = cpool.tile([P, HT, O_DIM], F16)
            nc.scalar.dma_start(w2t[:], w2t_d.ap().rearrange("(ht p) o -> p ht o", p=P))
            b2 = cpool.tile([P, OT], F32)
            nc.sync.dma_start(b2[:], b2_d.ap())

            fs_c = cc[:, 0:1]       # -7.09375 (floor spill)
            al_c = cc[:, 1:2]       # -1e-6 (Prelu alpha)
            b17_c = cc[:, 2:3]      # -17*ln2

            # PE warm-up: dummy matmuls on memset tiles while input DMAs
            # stream, so the HAM clock gate is released before real work.
            wu_a = cpool.tile([P, P], F16)
            nc.gpsimd.memset(wu_a[:], 0.0)
            wu_b = cpool.tile([P, NH], F16)
            nc.gpsimd.memset(wu_b[:], 0.0)
            ps_w = ppoolA.tile([P, B_LOC], F32, name="ps_warm", tag="psA")
            for w in range(3):
                nc.tensor.matmul(ps_w[:, :NH], lhsT=wu_a[:], rhs=wu_b[:],
                                 start=(w == 0), stop=(w == 2))

            s_all = spool.tile([P, HT, B_LOC], F16)

            # phase A matmuls for one (ht, bh) half: split matmul
            # w1.x = wh.xh + wh.xl + wl.xh accumulated in fp32 PSUM.
            def phase_a_half(ps, ht, bh):
                prods = [(w1th, xth), (w1th, xtl), (w1tl, xth)]
                nmm = len(prods) * KT
                i = 0
                for wsrc, xsrc in prods:
                    for kt in range(KT):
                        nc.tensor.matmul(
                            ps[:, bh * NH:(bh + 1) * NH],
                            lhsT=wsrc[:, ht, kt, :],
                            rhs=xsrc[:, bh, kt, :],
                            start=(i == 0),
                            stop=(i == nmm - 1),
                        )
                        i += 1

            def phase_a(ht):
                ps = ppoolA.tile([P, B_LOC], F32, name=f"ps_{ht}", tag="psA")
                for bh in range(2):
                    phase_a_half(ps, ht, bh)
                return ps

            def evict(ht, ps, sl=slice(0, B_LOC)):
                y = hpool.tile([P, B_LOC], F32, tag="y", name=f"y{ht}")
                nc.scalar.activation(y[:, sl], ps[:, sl], Prelu,
                                     bias=b1[:, ht:ht + 1], alpha=al_c)
                return y

            # chain is emitted in three skewed stages so no engine's
            # in-order queue ever waits cross-engine: front(t) does
            # r,n [DVE], x [ScalarE], f16 [DVE], nf [GpSimd]; mid(t) = B
            # [ScalarE] one tile later (nf long done); poly(t) [DVE] two
            # tiles later (B long done).
            xs, bs, nfs = {}, {}, {}

            # n, f16, nf, x, B are small integers / powers of two -- all
            # exactly representable in fp16, which halves the SBUF port
            # traffic these streams share with the GpSimd multiply.
            def front(ht, y):
                r = hpool.tile([P, B_LOC], F32, tag="r", name=f"r{ht}")
                nc.vector.reciprocal_approx_fast(out=r[:], in_=y[:])
                n = hpool.tile([P, B_LOC], F16, tag="n", name=f"n{ht}")
                nc.vector._custom_dve(MAGIC_N, out=n[:], in0=r[:],
                                      s0=CLAMP, s1=MAGIC, imm2=RND_OFF)
                x = hpool.tile([P, B_LOC], F16, tag="x", name=f"x{ht}",
                               bufs=4)
                nc.scalar.activation(x[:], n[:], Exp, scale=-LN2)
                f16 = hpool.tile([P, B_LOC], F16, tag="f16", name=f"f16_{ht}")
                nc.vector._custom_dve(FLOOR16F, out=f16[:], in0=n[:],
                                      in1=fs_c, s0=SEED16, s1=32.0,
                                      imm2=MAGIC27)
                nf = hpool.tile([P, B_LOC], F16, tag="nf", name=f"nf{ht}")
                nc.gpsimd.tensor_tensor(nf[:], f16[:], n[:],
                                        mybir.AluOpType.mult)
                xs[ht], nfs[ht] = x, nf

            def mid(ht):
                Bt = hpool.tile([P, B_LOC], F16, tag="B", name=f"B{ht}")
                nc.scalar.activation(Bt[:], nfs[ht][:], Exp,
                                     scale=LN2 / 16.0, bias=b17_c)
                bs[ht] = Bt

            def tail_poly(ht):
                nc.vector._custom_dve(POLY_MUL, out=s_all[:, ht, :],
                                      in0=xs[ht][:], in1=bs[ht][:])

            psC = [ppoolC.tile([P, B_LOC], F32, name=f"psc{ot}")
                   for ot in range(OT)]

            def phase_c(ht):
                for ot in range(OT):
                    for bh in range(2):
                        nc.tensor.matmul(
                            psC[ot][:, bh * NH:(bh + 1) * NH],
                            lhsT=w2t[:, ht, ot * P:(ot + 1) * P],
                            rhs=s_all[:, ht, bh * NH:(bh + 1) * NH],
                            start=(ht == 0),
                            stop=(ht == HT - 1),
                            skip_group_check=True,
                        )

            # software-pipelined schedule: PE runs A(ht) while the
            # ScalarE/DVE/GpSimd chain processes tile ht-1; phase-C matmuls
            # for tile k are emitted after chain(k+1) so the PE stream never
            # stalls more than one tile behind the elementwise pipeline.
            # The last tile drains in halves to cut the tail latency.
            out_sb = spool.tile([P, OT, B_LOC], F16)
            out_r = out_d.ap().rearrange("(ot p) b -> p ot b", p=P)

            ys = {}
            ps0 = phase_a(0)
            ys[0] = evict(0, ps0)
            for ht in range(1, HT):
                ps = phase_a(ht)
                ys[ht] = evict(ht, ps)
                front(ht - 1, ys[ht - 1])
                if ht >= 2:
                    mid(ht - 2)
                if ht >= 3:
                    tail_poly(ht - 3)
                if ht >= 4:
                    phase_c(ht - 4)
            front(HT - 1, ys[HT - 1])
            mid(HT - 2)
            tail_poly(HT - 3)
            phase_c(HT - 4)
            mid(HT - 1)
            tail_poly(HT - 2)
            phase_c(HT - 3)
            tail_poly(HT - 1)
            phase_c(HT - 2)
            phase_c(HT - 1)

            # evict + stream out per (ot, bh) quarter so the tail after the
            # final matmuls overlaps with the output DMAs
            for bh in range(2):
                sl = slice(bh * NH, (bh + 1) * NH)
                nc.scalar.activation(out_sb[:, 0, sl], psC[0][:, sl], ident,
                                     bias=b2[:, 0:1])
                nc.sync.dma_start(out_r[:, 0:1, sl], out_sb[:, 0:1, sl])
                nc.vector.tensor_scalar(out_sb[:, 1, sl], psC[1][:, sl],
                                        b2[:, 1:2], None,
                                        mybir.AluOpType.add)
                nc.scalar.dma_start(out_r[:, 1:2, sl], out_sb[:, 1:2, sl])

    nc.finalize()
    return nc


_NC_CACHE = None


def _get_nc() -> bacc.Bacc:
    global _NC_CACHE
    if _NC_CACHE is None:
        _NC_CACHE = _build_nc()
    return _NC_CACHE


# ------------------------------ entry point ----------------------------- #

def kernel(x, w1, b1, w2, b2, _trace=False, _tmpdir=None):
    x = np.ascontiguousarray(np.asarray(x, dtype=np.float32))
    w1 = np.ascontiguousarray(np.asarray(w1, dtype=np.float32))
    b1 = np.asarray(b1, dtype=np.float32)
    w2 = np.asarray(w2, dtype=np.float32)
    b2 = np.asarray(b2, dtype=np.float32)

    xt = np.ascontiguousarray(x.T)                               # [I, B]
    xth = xt.astype(np.float16)
    xtl = (xt - xth.astype(np.float32)).astype(np.float16)
    w1t = np.ascontiguousarray(w1.T)                             # [I, H]
    w1th = w1t.astype(np.float16)
    w1tl = (w1t - w1th.astype(np.float32)).astype(np.float16)

    # pack w1 splits to [P, ht, kt, 128]: w1p[p, ht, kt, j] = w1t[kt*128+p,
    # ht*128+j] -> flat [P, HT*KT*128] with contiguous per-(p, ht) chunks
    def _pack_w1(w):                                             # [I, H] f16
        v = w.reshape(KT, P, HT, P)                              # kt p ht j
        return np.ascontiguousarray(
            v.transpose(1, 2, 0, 3).reshape(P, HT * KT * P))

    w1thp = _pack_w1(w1th)
    w1tlp = _pack_w1(w1tl)
    b1c = np.ascontiguousarray(b1.reshape(HT, P).T)              # [P, HT]
    w2t = np.ascontiguousarray(w2.T.astype(np.float16))          # [H, O] fp16
    b2s = (np.float64(1.0) - 2.0 ** -T_STEPS) * b2.astype(np.float64)
    b2c = np.ascontiguousarray(b2s.astype(np.float32).reshape(OT, P).T)
    cc = np.ascontiguousarray(np.tile(
        np.array([[FLOOR_SPILL, PRELU_ALPHA, -17.0 * LN2]], dtype=np.float32),
        (P, 1)))

    # pack x splits to [P, bh, kt, 512] per core: xp[p, bh, kt, b] =
    # xt[kt*128+p, core*1024 + bh*512 + b]
    def _pack_x(xs):                                         # [I, 1024] f16
        v = xs.reshape(KT, P, 2, NH)                         # kt p bh b
        return np.ascontiguousarray(
            v.transpose(1, 2, 0, 3).reshape(P, 2 * KT * NH))

    in_maps = []
    for c in range(N_CORES):
        sl = slice(c * B_LOC, (c + 1) * B_LOC)
        in_maps.append({
            "xth": _pack_x(xth[:, sl]),
            "xtl": _pack_x(xtl[:, sl]),
            "w1th": w1thp,
            "w1tl": w1tlp,
            "b1c": b1c,
            "w2t": w2t,
            "b2c": b2c,
            "cc": cc,
        })

    nc = _get_nc()
    res = run_bass_kernel_spmd(
        nc, in_maps, core_ids=list(range(N_CORES)),
        trace=_trace, tmpdir=_tmpdir,
    )

    out = np.empty((B, O_DIM), dtype=np.float32)
    for c in range(N_CORES):
        out[c * B_LOC:(c + 1) * B_LOC, :] = \
            res.results[c]["outT"].astype(np.float32).T
    if _trace:
        kernel._last_results = res
    return out
